# revision 30
# baseline (speedup 1.0000x reference)
"""Trainium2 Bass kernel for a causal single-head attention module (v3).

reference computation (per batch b):
    q = x @ Wq; k = x @ Wk; v = x @ Wv          # [s, 128]
    att = softmax(mask(q @ k.T / sqrt(1024)))   # causal
    out = att @ v                               # [s, 128]

Shapes: x [4, 4096, 1024] f32, W* [1024, 128] f32.

Distribution: 8 NeuronCores, 2 per batch.  The 8 sequence blocks (512 rows
each) of a batch are split between its two cores: core 2b owns blocks
{1,3,5,7}, core 2b+1 owns {0,2,4,6}.  This interleaving balances the causal
triangle AND makes the per-core instruction graph identical (SPMD): every
core runs 4 q-tiles whose key-group counts are {2,4,6,8}; the odd core's
extra (non-causal) key group per tile is zeroed via a per-core input scalar.

Each core projects Q for its own rows and K^T/V^T for all 8 blocks
(K/V replicated within the pair; a pair AllGather was tried and lost --
the collective stack costs ~20us of serial latency).  V^T -> natural V
uses the DMA crossbar transpose (off the PE).  W DMAs go FIRST on the
SWDGE queue so the first projection matmul is not gated on mask setup.
xt lands in per-chunk SBUF tiles for precise DMA->PE gating.
Attention runs in the "St" orientation: St[k,q] = Kt_tile.T @ Qt so that
P^T = exp(St) is directly the stationary operand of the AV matmul.
Row sums use DVE partial adds + one ones-vector matmul per key group.
Normalisation and the final [dv, q] -> [q, dv] transpose happen on host
during unshard.
"""

import os
import ml_dtypes
import numpy as np

import concourse.bass as bass
import concourse.bacc as bacc
import concourse.mybir as mybir
import concourse.tile as tile
from concourse.bass_utils import run_bass_kernel_spmd

F32 = mybir.dt.float32
BF16 = mybir.dt.bfloat16

BATCH = 4
SEQ = 4096
EMB = 1024
DK = 128
P = 128
NCORES = 8
SCALE = 1.0 / float(np.sqrt(EMB))

NBLK = 8
HEAVY_BLOCKS = [1, 3, 5, 7]  # core 2b   (exact causal fit)
LIGHT_BLOCKS = [0, 2, 4, 6]  # core 2b+1 (one padded key-group per tile)


def build_nc(seq: int = SEQ):
    blk = seq // NBLK          # 512
    sub = blk // P             # 4 key subtiles per group
    kcols = 4 * blk            # own rows per core (2048)
    xcols = 8 * blk            # own + peer rows (K/V replicated)
    emb_c = EMB // P           # 8 contraction chunks
    nch = kcols // blk         # 4 projection column chunks of 512

    nc = bacc.Bacc("TRN2", target_bir_lowering=False, debug=False,
                   num_devices=NCORES)

    xt = nc.dram_tensor("xt", [EMB, xcols], BF16, kind="ExternalInput")
    wq = nc.dram_tensor("wq", [P, emb_c, DK], BF16, kind="ExternalInput")
    wk = nc.dram_tensor("wk", [P, emb_c, DK], BF16, kind="ExternalInput")
    wv = nc.dram_tensor("wv", [P, emb_c, DK], BF16, kind="ExternalInput")
    pad = nc.dram_tensor("pad", [P, 1], F32, kind="ExternalInput")
    out_o = nc.dram_tensor("out_o", [P, 4 * blk], F32, kind="ExternalOutput")
    out_s = nc.dram_tensor("out_s", [4, blk], F32, kind="ExternalOutput")

    with tile.TileContext(nc) as tc:
        with tc.tile_pool(name="persist", bufs=1) as persist:
            xt_t = [persist.tile([P, xcols], BF16, name=f"xt{c}")
                    for c in range(emb_c)]
            wq_sb = persist.tile([P, emb_c, DK], BF16)
            wk_sb = persist.tile([P, emb_c, DK], BF16)
            wv_sb = persist.tile([P, emb_c, DK], BF16)
            qt_sb = persist.tile([P, 4 * blk], BF16)
            kt_sb = persist.tile([P, 8 * blk], BF16)
            v_sb = persist.tile([P, 8 * sub, P], BF16)
            vt_all = persist.tile([P, xcols], BF16)
            dmask = persist.tile([P, sub, blk], BF16)
            ones_sb = persist.tile([P, 1], BF16)
            pad_sb = persist.tile([P, 1], F32)
            sums_sb = persist.tile([1, 4 * blk], F32)

            # ---- W tensors on the fast HWDGE rings, ahead of xt: they
            # gate the first projection matmuls (SWDGE desc-gen is slow).
            nc.sync.dma_start(wk_sb[:], wk.ap())
            nc.scalar.dma_start(wv_sb[:], wv.ap())
            # xt chunks: own-row halves first; per-chunk tiles for precise
            # DMA -> PE gating; two HWDGE rings.  wq rides the scalar ring
            # behind the own-half chunks (Q runs between the KV halves).
            # chunks 0 and 1 split across BOTH rings so the first
            # projection matmuls start as early as possible
            half_k = kcols // 2
            for c in (0, 1):
                e0, e1 = (nc.sync, nc.scalar) if c == 0 else (nc.scalar,
                                                              nc.sync)
                e0.dma_start(xt_t[c][:, 0:half_k],
                             xt.ap()[c * P:(c + 1) * P, 0:half_k])
                e1.dma_start(xt_t[c][:, half_k:kcols],
                             xt.ap()[c * P:(c + 1) * P, half_k:kcols])
            nc.scalar.dma_start(wq_sb[:], wq.ap())
            for c in range(2, emb_c):
                eng = nc.sync if c % 2 == 0 else nc.scalar
                eng.dma_start(xt_t[c][:, 0:kcols],
                              xt.ap()[c * P:(c + 1) * P, 0:kcols])
            for c in range(emb_c):
                eng = nc.sync if c % 2 == 0 else nc.scalar
                eng.dma_start(xt_t[c][:, kcols:xcols],
                              xt.ap()[c * P:(c + 1) * P, kcols:xcols])
            # constants / masks (after the weight DMAs)
            nc.gpsimd.memset(ones_sb[:], 1.0)
            nc.gpsimd.dma_start(pad_sb[:], pad.ap())
            nc.gpsimd.memset(dmask[:], 1.0)
            for j in range(sub):
                nc.gpsimd.affine_select(
                    out=dmask[:, j, :],
                    in_=dmask[:, j, :],
                    compare_op=mybir.AluOpType.is_ge,
                    fill=0.0,
                    base=-(j * P),
                    pattern=[[1, blk]],
                    channel_multiplier=-1,
                )

            # ---- K^T and V^T projections (both row-halves), chunk-outer
            # so the PE trails the xt DMA chunk arrivals.  V^T -> natural V
            # per 512-block via the DMA crossbar (off the PE).  The Q
            # projection runs between the two halves.
            def kv_half(half):
                lo = half * kcols
                with tc.tile_pool(name=f"kv_psum{half}", bufs=1,
                                  space="PSUM") as kvp:
                    k_ps = [kvp.tile([P, blk], F32, name=f"kps{half}_{n}")
                            for n in range(nch)]
                    v_ps = [kvp.tile([P, blk], F32, name=f"vps{half}_{n}")
                            for n in range(nch)]
                    for c in range(emb_c):
                        for n in range(nch):
                            nc.tensor.matmul(
                                k_ps[n][:], wk_sb[:, c, :],
                                xt_t[c][:, lo + n * blk:lo + (n + 1) * blk],
                                start=(c == 0), stop=(c == emb_c - 1))
                        for n in range(nch):
                            nc.tensor.matmul(
                                v_ps[n][:], wv_sb[:, c, :],
                                xt_t[c][:, lo + n * blk:lo + (n + 1) * blk],
                                start=(c == 0), stop=(c == emb_c - 1))
                    # PSUM -> SBUF copies spread over three engines so the
                    # bank-release chain is not serialized on the DVE
                    for n in range(nch):
                        dst = kt_sb[:, lo + n * blk:lo + (n + 1) * blk]
                        if n % 2 == 0:
                            nc.scalar.copy(dst, k_ps[n][:])
                        else:
                            nc.vector.tensor_copy(dst, k_ps[n][:])
                    for n in range(nch):
                        t = half * nch + n
                        dst = vt_all[:, t * blk:(t + 1) * blk]
                        if n % 2 == 0:
                            nc.scalar.copy(dst, v_ps[n][:])
                        else:
                            nc.vector.tensor_copy(dst, v_ps[n][:])
                        nc.sync.dma_start_transpose(
                            v_sb[:, t * sub:(t + 1) * sub, :], dst)

            # pass 1: K/V column-chunks 0,1 plus ALL of Q (8 PSUM banks
            # exactly), chunk-paced -- Q's matmuls soak up the PE idle time
            # while the own-row chunks stream in.
            with tc.tile_pool(name="kvq_psum", bufs=1, space="PSUM") as kvqp:
                k_ps = [kvqp.tile([P, blk], F32, name=f"kpsA{n}")
                        for n in range(2)]
                v_ps = [kvqp.tile([P, blk], F32, name=f"vpsA{n}")
                        for n in range(2)]
                q_ps = [kvqp.tile([P, blk], F32, name=f"qps{n}")
                        for n in range(nch)]
                for c in range(emb_c):
                    for n in range(2):
                        nc.tensor.matmul(
                            k_ps[n][:], wk_sb[:, c, :],
                            xt_t[c][:, n * blk:(n + 1) * blk],
                            start=(c == 0), stop=(c == emb_c - 1))
                    for n in range(2):
                        nc.tensor.matmul(
                            v_ps[n][:], wv_sb[:, c, :],
                            xt_t[c][:, n * blk:(n + 1) * blk],
                            start=(c == 0), stop=(c == emb_c - 1))
                    for n in range(nch):
                        nc.tensor.matmul(
                            q_ps[n][:], wq_sb[:, c, :],
                            xt_t[c][:, n * blk:(n + 1) * blk],
                            start=(c == 0), stop=(c == emb_c - 1))
                for n in range(2):
                    dst = kt_sb[:, n * blk:(n + 1) * blk]
                    if n % 2 == 0:
                        nc.scalar.copy(dst, k_ps[n][:])
                    else:
                        nc.vector.tensor_copy(dst, k_ps[n][:])
                for n in range(2):
                    dst = vt_all[:, n * blk:(n + 1) * blk]
                    if n % 2 == 0:
                        nc.scalar.copy(dst, v_ps[n][:])
                    else:
                        nc.vector.tensor_copy(dst, v_ps[n][:])
                    nc.sync.dma_start_transpose(
                        v_sb[:, n * sub:(n + 1) * sub, :], dst)
                for n in range(nch):
                    dst = qt_sb[:, n * blk:(n + 1) * blk]
                    if n % 2 == 0:
                        nc.scalar.copy(dst, q_ps[n][:])
                    else:
                        nc.vector.tensor_copy(dst, q_ps[n][:])

            # pass 2: K/V column-chunks 2,3 (all own chunks resident; dense)
            with tc.tile_pool(name="kv0b_psum", bufs=1, space="PSUM") as kvbp:
                k_ps2 = [kvbp.tile([P, blk], F32, name=f"kpsB{n}")
                         for n in range(2)]
                v_ps2 = [kvbp.tile([P, blk], F32, name=f"vpsB{n}")
                         for n in range(2)]
                for c in range(emb_c):
                    for n in range(2):
                        nc.tensor.matmul(
                            k_ps2[n][:], wk_sb[:, c, :],
                            xt_t[c][:, (2 + n) * blk:(3 + n) * blk],
                            start=(c == 0), stop=(c == emb_c - 1))
                    for n in range(2):
                        nc.tensor.matmul(
                            v_ps2[n][:], wv_sb[:, c, :],
                            xt_t[c][:, (2 + n) * blk:(3 + n) * blk],
                            start=(c == 0), stop=(c == emb_c - 1))
                for n in range(2):
                    dst = kt_sb[:, (2 + n) * blk:(3 + n) * blk]
                    if n % 2 == 0:
                        nc.scalar.copy(dst, k_ps2[n][:])
                    else:
                        nc.vector.tensor_copy(dst, k_ps2[n][:])
                for n in range(2):
                    dst = vt_all[:, (2 + n) * blk:(3 + n) * blk]
                    if n % 2 == 0:
                        nc.scalar.copy(dst, v_ps2[n][:])
                    else:
                        nc.vector.tensor_copy(dst, v_ps2[n][:])
                    nc.sync.dma_start_transpose(
                        v_sb[:, (2 + n) * sub:(3 + n) * sub, :], dst)

            kv_half(1)

            # ---- attention: one pass per q-tile, biggest tile first ----
            halves = 2
            hs = sub // halves
            with (
                tc.tile_pool(name="st_psum", bufs=2, space="PSUM") as stp,
                tc.tile_pool(name="ot_psum", bufs=2, space="PSUM") as otp,
                tc.tile_pool(name="sum_psum", bufs=2, space="PSUM") as smp,
                tc.tile_pool(name="pt_pool", bufs=4) as ptp,
                tc.tile_pool(name="acc_pool", bufs=4) as accp,
                tc.tile_pool(name="ot_sb_pool", bufs=2) as osp,
            ):
                for i in (3, 2, 1, 0):
                    slots = list(range(0, i + 1)) + list(range(4, 5 + i))
                    ot = otp.tile([P, blk], F32, tag="ot", name=f"ot_{i}")
                    sm = smp.tile([1, blk], F32, tag="sm", name=f"sm_{i}")
                    n_mm = 2 * (i + 1) * sub
                    mm = 0
                    qs = qt_sb[:, i * blk:(i + 1) * blk]
                    accs = []
                    sm_n = 0
                    # per-tile sum batching: fewer ones-matmuls on the PE,
                    # tree-reduced on the DVE; last tile stays unbatched so
                    # the kernel tail is short
                    plan = {3: [4, 4], 2: [4, 2], 1: [4], 0: [1, 1]}[i]
                    bi = 0
                    for si, s in enumerate(slots):
                        pts = []
                        diag = (s == i)
                        for h in range(halves):
                            st = stp.tile([P, hs * blk], F32, tag="st")
                            for j in range(hs):
                                jj = h * hs + j
                                # on the diagonal group only q >= key is
                                # live; skip the upper-triangle columns
                                off = jj * P if diag else 0
                                nc.tensor.matmul(
                                    st[:, j * blk + off:(j + 1) * blk],
                                    kt_sb[:, s * blk + jj * P:
                                          s * blk + (jj + 1) * P],
                                    qs[:, off:blk],
                                    start=True, stop=True)
                            pt = ptp.tile([P, hs * blk], BF16, tag="pt")
                            if diag:
                                for j in range(hs):
                                    off = (h * hs + j) * P
                                    nc.scalar.activation(
                                        pt[:, j * blk + off:(j + 1) * blk],
                                        st[:, j * blk + off:(j + 1) * blk],
                                        mybir.ActivationFunctionType.Exp,
                                        bias=0.0, scale=SCALE)
                                # the tri mask also zeroes the stale
                                # (skipped) upper-triangle region of pt
                                nc.vector.tensor_tensor(
                                    pt[:], pt[:],
                                    dmask[:, h * hs:(h + 1) * hs, :]
                                    .rearrange("p s b -> p (s b)"),
                                    mybir.AluOpType.mult)
                            else:
                                nc.scalar.activation(
                                    pt[:], st[:],
                                    mybir.ActivationFunctionType.Exp,
                                    bias=0.0, scale=SCALE)
                            if s == 4 + i:  # pad group (zeroed on light)
                                nc.vector.tensor_scalar_mul(
                                    pt[:], pt[:], pad_sb[:, 0:1])
                            pts.append(pt)
                        # Ot accumulation AFTER both halves' St+exp: keeps
                        # the scalar engine's exp stream gapless (exp h1
                        # only needs St h1, not AV h0)
                        for h in range(halves):
                            pt = pts[h]
                            for j in range(hs):
                                jj = h * hs + j
                                off = jj * P if diag else 0
                                nc.tensor.matmul(
                                    ot[:, off:blk],
                                    v_sb[:, s * sub + jj, :],
                                    pt[:, j * blk + off:(j + 1) * blk],
                                    start=(mm == 0),
                                    stop=(mm == n_mm - 1))
                                mm += 1
                        # row sums: DVE partial adds reduce each group to a
                        # [P, blk] tile; consecutive group PAIRS share one
                        # ones-matmul (PSUM-accumulated across the tile)
                        acc = accp.tile([P, blk], BF16, tag="acc")
                        h0, h1 = pts
                        nc.vector.tensor_tensor(
                            acc[:], h0[:, 0:blk], h0[:, blk:2 * blk],
                            mybir.AluOpType.add)
                        tmp = accp.tile([P, blk], BF16, tag="acc2")
                        nc.vector.tensor_tensor(
                            tmp[:], h1[:, 0:blk], h1[:, blk:2 * blk],
                            mybir.AluOpType.add)
                        nc.vector.tensor_tensor(
                            acc[:], acc[:], tmp[:], mybir.AluOpType.add)
                        accs.append(acc)
                        if len(accs) == plan[bi]:
                            for a in accs[1:]:
                                nc.vector.tensor_tensor(
                                    accs[0][:], accs[0][:], a[:],
                                    mybir.AluOpType.add)
                            nc.tensor.matmul(sm[:], ones_sb[:, 0:1],
                                             accs[0][:],
                                             start=(sm_n == 0),
                                             stop=(si == len(slots) - 1))
                            sm_n += 1
                            bi += 1
                            accs = []
                    nc.vector.tensor_copy(
                        sums_sb[0:1, i * blk:(i + 1) * blk], sm[:])
                    ot_out = osp.tile([P, blk], F32, tag="ot_sb")
                    nc.vector.tensor_copy(ot_out[:], ot[:])
                    nc.sync.dma_start(out_o.ap()[:, i * blk:(i + 1) * blk],
                                      ot_out[:])
                    nc.sync.dma_start(out_s.ap()[i:i + 1, :],
                                      sums_sb[0:1, i * blk:(i + 1) * blk])

    nc.compile()
    return nc


_NC_CACHE = {}


def _get_nc(seq: int):
    if seq not in _NC_CACHE:
        _NC_CACHE[seq] = build_nc(seq)
    return _NC_CACHE[seq]


def make_in_maps(x, Wq, Wk, Wv, seq=None):
    """Host-side sharding: build the 8 per-core input maps."""
    x = np.asarray(x, dtype=np.float32)
    Wq = np.asarray(Wq, dtype=np.float32)
    Wk = np.asarray(Wk, dtype=np.float32)
    Wv = np.asarray(Wv, dtype=np.float32)
    seq = seq or x.shape[1]
    blk = seq // NBLK
    in_maps = []

    def warr(W):
        # [1024, 128] -> [P, emb_chunks, 128] so the device DMA is contiguous
        return np.ascontiguousarray(
            W.reshape(-1, P, DK).transpose(1, 0, 2)).astype(ml_dtypes.bfloat16)

    warrs = {"wq": warr(Wq), "wk": warr(Wk), "wv": warr(Wv)}
    for core in range(NCORES):
        b, h = core // 2, core % 2
        blocks = HEAVY_BLOCKS if h == 0 else LIGHT_BLOCKS
        rows = np.concatenate(
            [np.arange(g * blk, (g + 1) * blk) for g in blocks])
        peer_blocks = LIGHT_BLOCKS if h == 0 else HEAVY_BLOCKS
        rows_peer = np.concatenate(
            [np.arange(g * blk, (g + 1) * blk) for g in peer_blocks])
        all_rows = np.concatenate([rows, rows_peer])
        xt = np.ascontiguousarray(x[b].T[:, all_rows]).astype(
            ml_dtypes.bfloat16)
        padv = np.full((P, 1), 1.0 if h == 0 else 0.0, dtype=np.float32)
        in_maps.append({
            "xt": xt,
            "pad": padv,
            **warrs,
        })
    return in_maps


def unshard(results, seq=None, batch=BATCH):
    seq = seq or SEQ
    blk = seq // NBLK
    out = np.empty((batch, seq, DK), dtype=np.float32)
    for core in range(NCORES):
        b, h = core // 2, core % 2
        blocks = HEAVY_BLOCKS if h == 0 else LIGHT_BLOCKS
        oo = np.asarray(results[core]["out_o"])  # [128, 4*blk]
        ss = np.asarray(results[core]["out_s"])  # [4, blk]
        for i, g in enumerate(blocks):
            o_cols = oo[:, i * blk:(i + 1) * blk]        # [dv, blk]
            out[b, g * blk:(g + 1) * blk, :] = (o_cols / ss[i][None, :]).T
    return out


LAST_EXEC_NS = None
LAST_RESULTS = None


def kernel(x, Wq, Wk, Wv):
    global LAST_EXEC_NS, LAST_RESULTS
    x = np.asarray(x, dtype=np.float32)
    seq = x.shape[1]
    nc = _get_nc(seq)
    in_maps = make_in_maps(x, Wq, Wk, Wv, seq)
    trace = bool(os.environ.get("BASS_KERNEL_TRACE"))
    res = run_bass_kernel_spmd(nc, in_maps, core_ids=list(range(NCORES)),
                               trace=trace)
    LAST_EXEC_NS = res.exec_time_ns
    LAST_RESULTS = res
    return unshard(res.results, seq, x.shape[0])


if __name__ == "__main__":
    rng = np.random.default_rng(0)
    x = rng.standard_normal((BATCH, SEQ, EMB), dtype=np.float32)
    Wq = rng.standard_normal((EMB, DK), dtype=np.float32) / 32
    Wk = rng.standard_normal((EMB, DK), dtype=np.float32) / 32
    Wv = rng.standard_normal((EMB, DK), dtype=np.float32) / 32
    out = kernel(x, Wq, Wk, Wv)
    print("out", out.shape, out.dtype, "exec_ns", LAST_EXEC_NS)


# revision 32
# speedup vs baseline: 1.0506x; 1.0506x over previous
"""Trainium2 Bass kernel for a causal single-head attention module (v3).

reference computation (per batch b):
    q = x @ Wq; k = x @ Wk; v = x @ Wv          # [s, 128]
    att = softmax(mask(q @ k.T / sqrt(1024)))   # causal
    out = att @ v                               # [s, 128]

Shapes: x [4, 4096, 1024] f32, W* [1024, 128] f32.

Distribution: 8 NeuronCores, 2 per batch.  The 8 sequence blocks (512 rows
each) of a batch are split between its two cores: core 2b owns blocks
{1,3,5,7}, core 2b+1 owns {0,2,4,6}.  This interleaving balances the causal
triangle AND makes the per-core instruction graph identical (SPMD): every
core runs 4 q-tiles whose key-group counts are {2,4,6,8}; the odd core's
extra (non-causal) key group per tile is zeroed via a per-core input scalar.

Each core projects Q for its own rows and K^T/V^T for all 8 blocks
(K/V replicated within the pair; a pair AllGather was tried and lost --
the collective stack costs ~20us of serial latency).  V^T -> natural V
uses the DMA crossbar transpose (off the PE).  W DMAs go FIRST on the
SWDGE queue so the first projection matmul is not gated on mask setup.
xt lands in per-chunk SBUF tiles for precise DMA->PE gating.
Attention runs in the "St" orientation: St[k,q] = Kt_tile.T @ Qt so that
P^T = exp(St) is directly the stationary operand of the AV matmul.
Row sums use DVE partial adds + one ones-vector matmul per key group.
Normalisation and the final [dv, q] -> [q, dv] transpose happen on host
during unshard.
"""

import os
import ml_dtypes
import numpy as np

import concourse.bass as bass
import concourse.bacc as bacc
import concourse.mybir as mybir
import concourse.tile as tile
from concourse.bass_utils import run_bass_kernel_spmd

F32 = mybir.dt.float32
BF16 = mybir.dt.bfloat16

BATCH = 4
SEQ = 4096
EMB = 1024
DK = 128
P = 128
NCORES = 8
SCALE = 1.0 / float(np.sqrt(EMB))

NBLK = 8
HEAVY_BLOCKS = [1, 3, 5, 7]  # core 2b   (exact causal fit)
LIGHT_BLOCKS = [0, 2, 4, 6]  # core 2b+1 (one padded key-group per tile)


def build_nc(seq: int = SEQ):
    blk = seq // NBLK          # 512
    sub = blk // P             # 4 key subtiles per group
    kcols = 4 * blk            # own rows per core (2048)
    xcols = 8 * blk            # own + peer rows (K/V replicated)
    emb_c = EMB // P           # 8 contraction chunks
    nch = kcols // blk         # 4 projection column chunks of 512

    nc = bacc.Bacc("TRN2", target_bir_lowering=False, debug=False,
                   num_devices=NCORES)

    xt = nc.dram_tensor("xt", [EMB, xcols], BF16, kind="ExternalInput")
    wq = nc.dram_tensor("wq", [P, emb_c, DK], BF16, kind="ExternalInput")
    wk = nc.dram_tensor("wk", [P, emb_c, DK], BF16, kind="ExternalInput")
    wv = nc.dram_tensor("wv", [P, emb_c, DK], BF16, kind="ExternalInput")
    pad = nc.dram_tensor("pad", [P, 1], F32, kind="ExternalInput")
    out_o = nc.dram_tensor("out_o", [P, 4 * blk], F32, kind="ExternalOutput")
    out_s = nc.dram_tensor("out_s", [4, blk], F32, kind="ExternalOutput")

    with tile.TileContext(nc) as tc:
        with tc.tile_pool(name="persist", bufs=1) as persist:
            xt_t = [persist.tile([P, xcols], BF16, name=f"xt{c}")
                    for c in range(emb_c)]
            wq_sb = persist.tile([P, emb_c, DK], BF16)
            wk_sb = persist.tile([P, emb_c, DK], BF16)
            wv_sb = persist.tile([P, emb_c, DK], BF16)
            qt_sb = persist.tile([P, 4 * blk], BF16)
            kt_sb = persist.tile([P, 8 * blk], BF16)
            v_sb = persist.tile([P, 8 * sub, P], BF16)
            vt_all = persist.tile([P, xcols], BF16)
            dmask = persist.tile([P, sub, blk], BF16)
            ones_sb = persist.tile([P, 1], BF16)
            pad_sb = persist.tile([P, 1], F32)
            sums_sb = persist.tile([1, 4 * blk], F32)

            # ---- W tensors on the fast HWDGE rings, ahead of xt: they
            # gate the first projection matmuls (SWDGE desc-gen is slow).
            nc.sync.dma_start(wk_sb[:], wk.ap())
            nc.scalar.dma_start(wv_sb[:], wv.ap())
            # xt chunks: own-row halves first; per-chunk tiles for precise
            # DMA -> PE gating; two HWDGE rings.  wq rides the scalar ring
            # behind the own-half chunks (Q runs between the KV halves).
            # chunks 0 and 1 split across BOTH rings so the first
            # projection matmuls start as early as possible
            half_k = kcols // 2
            for c in (0, 1):
                e0, e1 = (nc.sync, nc.scalar) if c == 0 else (nc.scalar,
                                                              nc.sync)
                e0.dma_start(xt_t[c][:, 0:half_k],
                             xt.ap()[c * P:(c + 1) * P, 0:half_k])
                e1.dma_start(xt_t[c][:, half_k:kcols],
                             xt.ap()[c * P:(c + 1) * P, half_k:kcols])
            for c in range(2, emb_c):
                eng = nc.sync if c % 2 == 0 else nc.scalar
                eng.dma_start(xt_t[c][:, 0:kcols],
                              xt.ap()[c * P:(c + 1) * P, 0:kcols])
            nc.scalar.dma_start(wq_sb[:], wq.ap())
            for c in range(emb_c):
                eng = nc.sync if c % 2 == 0 else nc.scalar
                eng.dma_start(xt_t[c][:, kcols:xcols],
                              xt.ap()[c * P:(c + 1) * P, kcols:xcols])
            # constants / masks (after the weight DMAs)
            nc.sync.dma_start(pad_sb[:], pad.ap())
            nc.gpsimd.memset(ones_sb[:], 1.0)
            nc.gpsimd.memset(dmask[:], 1.0)
            for j in range(sub):
                nc.gpsimd.affine_select(
                    out=dmask[:, j, :],
                    in_=dmask[:, j, :],
                    compare_op=mybir.AluOpType.is_ge,
                    fill=0.0,
                    base=-(j * P),
                    pattern=[[1, blk]],
                    channel_multiplier=-1,
                )

            # ---- K^T and V^T projections (both row-halves), chunk-outer
            # so the PE trails the xt DMA chunk arrivals.  V^T -> natural V
            # per 512-block via the DMA crossbar (off the PE).  The Q
            # projection runs between the two halves.
            def kv_half(half):
                lo = half * kcols
                with tc.tile_pool(name=f"kv_psum{half}", bufs=1,
                                  space="PSUM") as kvp:
                    k_ps = [kvp.tile([P, blk], F32, name=f"kps{half}_{n}")
                            for n in range(nch)]
                    v_ps = [kvp.tile([P, blk], F32, name=f"vps{half}_{n}")
                            for n in range(nch)]
                    for c in range(emb_c):
                        for n in range(nch):
                            nc.tensor.matmul(
                                k_ps[n][:], wk_sb[:, c, :],
                                xt_t[c][:, lo + n * blk:lo + (n + 1) * blk],
                                start=(c == 0), stop=(c == emb_c - 1))
                        for n in range(nch):
                            nc.tensor.matmul(
                                v_ps[n][:], wv_sb[:, c, :],
                                xt_t[c][:, lo + n * blk:lo + (n + 1) * blk],
                                start=(c == 0), stop=(c == emb_c - 1))
                    # PSUM -> SBUF copies spread over three engines so the
                    # bank-release chain is not serialized on the DVE
                    for n in range(nch):
                        dst = kt_sb[:, lo + n * blk:lo + (n + 1) * blk]
                        if n % 2 == 0:
                            nc.scalar.copy(dst, k_ps[n][:])
                        else:
                            nc.vector.tensor_copy(dst, k_ps[n][:])
                    for n in range(nch):
                        t = half * nch + n
                        dst = vt_all[:, t * blk:(t + 1) * blk]
                        if n % 2 == 0:
                            nc.scalar.copy(dst, v_ps[n][:])
                        else:
                            nc.vector.tensor_copy(dst, v_ps[n][:])
                        nc.sync.dma_start_transpose(
                            v_sb[:, t * sub:(t + 1) * sub, :], dst)

            kv_half(0)

            # ---- Q^T projection (own rows only), between the KV halves:
            # it needs only own-row chunks, filling the peer-chunk DMA wait.
            with tc.tile_pool(name="q_psum", bufs=2, space="PSUM") as qp:
                for n in range(nch):
                    ps = qp.tile([P, blk], F32, tag="qproj")
                    for c in range(emb_c):
                        nc.tensor.matmul(ps[:], wq_sb[:, c, :],
                                         xt_t[c][:, n * blk:(n + 1) * blk],
                                         start=(c == 0),
                                         stop=(c == emb_c - 1))
                    if n % 2 == 0:
                        nc.scalar.copy(qt_sb[:, n * blk:(n + 1) * blk],
                                       ps[:])
                    else:
                        nc.vector.tensor_copy(
                            qt_sb[:, n * blk:(n + 1) * blk], ps[:])

            kv_half(1)

            # ---- attention: one pass per q-tile, biggest tile first ----
            halves = 2
            hs = sub // halves
            with (
                tc.tile_pool(name="st_psum", bufs=2, space="PSUM") as stp,
                tc.tile_pool(name="ot_psum", bufs=2, space="PSUM") as otp,
                tc.tile_pool(name="sum_psum", bufs=2, space="PSUM") as smp,
                tc.tile_pool(name="pt_pool", bufs=4) as ptp,
                tc.tile_pool(name="acc_pool", bufs=4) as accp,
                tc.tile_pool(name="ot_sb_pool", bufs=2) as osp,
            ):
                for i in (3, 2, 1, 0):
                    slots = list(range(0, i + 1)) + list(range(4, 5 + i))
                    ot = otp.tile([P, blk], F32, tag="ot", name=f"ot_{i}")
                    sm = smp.tile([1, blk], F32, tag="sm", name=f"sm_{i}")
                    n_mm = 2 * (i + 1) * sub
                    mm = 0
                    qs = qt_sb[:, i * blk:(i + 1) * blk]
                    accs = []
                    sm_n = 0
                    # per-tile sum batching: fewer ones-matmuls on the PE,
                    # tree-reduced on the DVE; last tile stays unbatched so
                    # the kernel tail is short
                    plan = {3: [4, 4], 2: [4, 2], 1: [4], 0: [1, 1]}[i]
                    bi = 0
                    for si, s in enumerate(slots):
                        pts = []
                        diag = (s == i)
                        for h in range(halves):
                            st = stp.tile([P, hs * blk], F32, tag="st")
                            for j in range(hs):
                                jj = h * hs + j
                                # on the diagonal group only q >= key is
                                # live; skip the upper-triangle columns
                                off = jj * P if diag else 0
                                nc.tensor.matmul(
                                    st[:, j * blk + off:(j + 1) * blk],
                                    kt_sb[:, s * blk + jj * P:
                                          s * blk + (jj + 1) * P],
                                    qs[:, off:blk],
                                    start=True, stop=True)
                            pt = ptp.tile([P, hs * blk], BF16, tag="pt")
                            if diag:
                                for j in range(hs):
                                    off = (h * hs + j) * P
                                    nc.scalar.activation(
                                        pt[:, j * blk + off:(j + 1) * blk],
                                        st[:, j * blk + off:(j + 1) * blk],
                                        mybir.ActivationFunctionType.Exp,
                                        bias=0.0, scale=SCALE)
                                # the tri mask also zeroes the stale
                                # (skipped) upper-triangle region of pt
                                nc.vector.tensor_tensor(
                                    pt[:], pt[:],
                                    dmask[:, h * hs:(h + 1) * hs, :]
                                    .rearrange("p s b -> p (s b)"),
                                    mybir.AluOpType.mult)
                            else:
                                nc.scalar.activation(
                                    pt[:], st[:],
                                    mybir.ActivationFunctionType.Exp,
                                    bias=0.0, scale=SCALE)
                            if s == 4 + i:  # pad group (zeroed on light)
                                nc.vector.tensor_scalar_mul(
                                    pt[:], pt[:], pad_sb[:, 0:1])
                            pts.append(pt)
                        # Ot accumulation AFTER both halves' St+exp: keeps
                        # the scalar engine's exp stream gapless (exp h1
                        # only needs St h1, not AV h0)
                        for h in range(halves):
                            pt = pts[h]
                            for j in range(hs):
                                jj = h * hs + j
                                off = jj * P if diag else 0
                                nc.tensor.matmul(
                                    ot[:, off:blk],
                                    v_sb[:, s * sub + jj, :],
                                    pt[:, j * blk + off:(j + 1) * blk],
                                    start=(mm == 0),
                                    stop=(mm == n_mm - 1))
                                mm += 1
                        # row sums: DVE partial adds reduce each group to a
                        # [P, blk] tile; consecutive group PAIRS share one
                        # ones-matmul (PSUM-accumulated across the tile)
                        acc = accp.tile([P, blk], BF16, tag="acc")
                        h0, h1 = pts
                        nc.vector.tensor_tensor(
                            acc[:], h0[:, 0:blk], h0[:, blk:2 * blk],
                            mybir.AluOpType.add)
                        tmp = accp.tile([P, blk], BF16, tag="acc2")
                        nc.vector.tensor_tensor(
                            tmp[:], h1[:, 0:blk], h1[:, blk:2 * blk],
                            mybir.AluOpType.add)
                        nc.vector.tensor_tensor(
                            acc[:], acc[:], tmp[:], mybir.AluOpType.add)
                        accs.append(acc)
                        if len(accs) == plan[bi]:
                            for a in accs[1:]:
                                nc.vector.tensor_tensor(
                                    accs[0][:], accs[0][:], a[:],
                                    mybir.AluOpType.add)
                            nc.tensor.matmul(sm[:], ones_sb[:, 0:1],
                                             accs[0][:],
                                             start=(sm_n == 0),
                                             stop=(si == len(slots) - 1))
                            sm_n += 1
                            bi += 1
                            accs = []
                    nc.vector.tensor_copy(
                        sums_sb[0:1, i * blk:(i + 1) * blk], sm[:])
                    ot_out = osp.tile([P, blk], F32, tag="ot_sb")
                    if i == 0:
                        hb = blk // 2
                        nc.scalar.copy(ot_out[:, 0:hb], ot[:, 0:hb])
                        nc.sync.dma_start(
                            out_o.ap()[:, i * blk:i * blk + hb],
                            ot_out[:, 0:hb])
                        nc.vector.tensor_copy(ot_out[:, hb:blk],
                                              ot[:, hb:blk])
                        nc.sync.dma_start(
                            out_o.ap()[:, i * blk + hb:(i + 1) * blk],
                            ot_out[:, hb:blk])
                    else:
                        nc.vector.tensor_copy(ot_out[:], ot[:])
                        nc.sync.dma_start(
                            out_o.ap()[:, i * blk:(i + 1) * blk], ot_out[:])
                    nc.sync.dma_start(out_s.ap()[i:i + 1, :],
                                      sums_sb[0:1, i * blk:(i + 1) * blk])

    nc.compile()
    return nc


_NC_CACHE = {}


def _get_nc(seq: int):
    if seq not in _NC_CACHE:
        _NC_CACHE[seq] = build_nc(seq)
    return _NC_CACHE[seq]


def make_in_maps(x, Wq, Wk, Wv, seq=None):
    """Host-side sharding: build the 8 per-core input maps."""
    x = np.asarray(x, dtype=np.float32)
    Wq = np.asarray(Wq, dtype=np.float32)
    Wk = np.asarray(Wk, dtype=np.float32)
    Wv = np.asarray(Wv, dtype=np.float32)
    seq = seq or x.shape[1]
    blk = seq // NBLK
    in_maps = []

    def warr(W):
        # [1024, 128] -> [P, emb_chunks, 128] so the device DMA is contiguous
        return np.ascontiguousarray(
            W.reshape(-1, P, DK).transpose(1, 0, 2)).astype(ml_dtypes.bfloat16)

    warrs = {"wq": warr(Wq), "wk": warr(Wk), "wv": warr(Wv)}
    for core in range(NCORES):
        b, h = core // 2, core % 2
        blocks = HEAVY_BLOCKS if h == 0 else LIGHT_BLOCKS
        rows = np.concatenate(
            [np.arange(g * blk, (g + 1) * blk) for g in blocks])
        peer_blocks = LIGHT_BLOCKS if h == 0 else HEAVY_BLOCKS
        rows_peer = np.concatenate(
            [np.arange(g * blk, (g + 1) * blk) for g in peer_blocks])
        all_rows = np.concatenate([rows, rows_peer])
        xt = np.ascontiguousarray(x[b].T[:, all_rows]).astype(
            ml_dtypes.bfloat16)
        padv = np.full((P, 1), 1.0 if h == 0 else 0.0, dtype=np.float32)
        in_maps.append({
            "xt": xt,
            "pad": padv,
            **warrs,
        })
    return in_maps


def unshard(results, seq=None, batch=BATCH):
    seq = seq or SEQ
    blk = seq // NBLK
    out = np.empty((batch, seq, DK), dtype=np.float32)
    for core in range(NCORES):
        b, h = core // 2, core % 2
        blocks = HEAVY_BLOCKS if h == 0 else LIGHT_BLOCKS
        oo = np.asarray(results[core]["out_o"])  # [128, 4*blk]
        ss = np.asarray(results[core]["out_s"])  # [4, blk]
        for i, g in enumerate(blocks):
            o_cols = oo[:, i * blk:(i + 1) * blk]        # [dv, blk]
            out[b, g * blk:(g + 1) * blk, :] = (o_cols / ss[i][None, :]).T
    return out


LAST_EXEC_NS = None
LAST_RESULTS = None


def kernel(x, Wq, Wk, Wv):
    global LAST_EXEC_NS, LAST_RESULTS
    x = np.asarray(x, dtype=np.float32)
    seq = x.shape[1]
    nc = _get_nc(seq)
    in_maps = make_in_maps(x, Wq, Wk, Wv, seq)
    trace = bool(os.environ.get("BASS_KERNEL_TRACE"))
    res = run_bass_kernel_spmd(nc, in_maps, core_ids=list(range(NCORES)),
                               trace=trace)
    LAST_EXEC_NS = res.exec_time_ns
    LAST_RESULTS = res
    return unshard(res.results, seq, x.shape[0])


if __name__ == "__main__":
    rng = np.random.default_rng(0)
    x = rng.standard_normal((BATCH, SEQ, EMB), dtype=np.float32)
    Wq = rng.standard_normal((EMB, DK), dtype=np.float32) / 32
    Wk = rng.standard_normal((EMB, DK), dtype=np.float32) / 32
    Wv = rng.standard_normal((EMB, DK), dtype=np.float32) / 32
    out = kernel(x, Wq, Wk, Wv)
    print("out", out.shape, out.dtype, "exec_ns", LAST_EXEC_NS)


# revision 34
# speedup vs baseline: 1.0845x; 1.0322x over previous
"""Trainium2 Bass kernel for a causal single-head attention module (v3).

reference computation (per batch b):
    q = x @ Wq; k = x @ Wk; v = x @ Wv          # [s, 128]
    att = softmax(mask(q @ k.T / sqrt(1024)))   # causal
    out = att @ v                               # [s, 128]

Shapes: x [4, 4096, 1024] f32, W* [1024, 128] f32.

Distribution: 8 NeuronCores, 2 per batch.  The 8 sequence blocks (512 rows
each) of a batch are split between its two cores: core 2b owns blocks
{1,3,5,7}, core 2b+1 owns {0,2,4,6}.  This interleaving balances the causal
triangle AND makes the per-core instruction graph identical (SPMD): every
core runs 4 q-tiles whose key-group counts are {2,4,6,8}; the odd core's
extra (non-causal) key group per tile is zeroed via a per-core input scalar.

Each core projects Q for its own rows and K^T/V^T for all 8 blocks
(K/V replicated within the pair; a pair AllGather was tried and lost --
the collective stack costs ~20us of serial latency).  V^T -> natural V
uses the DMA crossbar transpose (off the PE).  W DMAs go FIRST on the
SWDGE queue so the first projection matmul is not gated on mask setup.
xt lands in per-chunk SBUF tiles for precise DMA->PE gating.
Attention runs in the "St" orientation: St[k,q] = Kt_tile.T @ Qt so that
P^T = exp(St) is directly the stationary operand of the AV matmul.
Row sums use DVE partial adds + one ones-vector matmul per key group.
Normalisation and the final [dv, q] -> [q, dv] transpose happen on host
during unshard.
"""

import os
import ml_dtypes
import numpy as np

import concourse.bass as bass
import concourse.bacc as bacc
import concourse.mybir as mybir
import concourse.tile as tile
from concourse.bass_utils import run_bass_kernel_spmd

F32 = mybir.dt.float32
BF16 = mybir.dt.bfloat16

BATCH = 4
SEQ = 4096
EMB = 1024
DK = 128
P = 128
NCORES = 8
SCALE = 1.0 / float(np.sqrt(EMB))

NBLK = 8
HEAVY_BLOCKS = [1, 3, 5, 7]  # core 2b   (exact causal fit)
LIGHT_BLOCKS = [0, 2, 4, 6]  # core 2b+1 (one padded key-group per tile)


def build_nc(seq: int = SEQ):
    blk = seq // NBLK          # 512
    sub = blk // P             # 4 key subtiles per group
    kcols = 4 * blk            # own rows per core (2048)
    xcols = 8 * blk            # own + peer rows (K/V replicated)
    emb_c = EMB // P           # 8 contraction chunks
    nch = kcols // blk         # 4 projection column chunks of 512

    nc = bacc.Bacc("TRN2", target_bir_lowering=False, debug=False,
                   num_devices=NCORES)

    xt = nc.dram_tensor("xt", [EMB, xcols], BF16, kind="ExternalInput")
    wq = nc.dram_tensor("wq", [P, emb_c, DK], BF16, kind="ExternalInput")
    wk = nc.dram_tensor("wk", [P, emb_c, DK], BF16, kind="ExternalInput")
    wv = nc.dram_tensor("wv", [P, emb_c, DK], BF16, kind="ExternalInput")
    pad = nc.dram_tensor("pad", [P, 1], F32, kind="ExternalInput")
    out_o = nc.dram_tensor("out_o", [P, 4 * blk], F32, kind="ExternalOutput")
    out_s = nc.dram_tensor("out_s", [4, blk], F32, kind="ExternalOutput")

    with tile.TileContext(nc) as tc:
        with tc.tile_pool(name="persist", bufs=1) as persist:
            xt_t = [persist.tile([P, xcols], BF16, name=f"xt{c}")
                    for c in range(emb_c)]
            wq_sb = persist.tile([P, emb_c, DK], BF16)
            wk_sb = persist.tile([P, emb_c, DK], BF16)
            wv_sb = persist.tile([P, emb_c, DK], BF16)
            qt_sb = persist.tile([P, 4 * blk], BF16)
            kt_sb = persist.tile([P, 8 * blk], BF16)
            v_sb = persist.tile([P, 8 * sub, P], BF16)
            vt_all = persist.tile([P, xcols], BF16)
            dmask = persist.tile([P, sub, blk], BF16)
            ones_sb = persist.tile([P, 1], BF16)
            pad_sb = persist.tile([P, 1], F32)
            sums_sb = persist.tile([1, 4 * blk], F32)

            # ---- W tensors on the fast HWDGE rings, ahead of xt: they
            # gate the first projection matmuls (SWDGE desc-gen is slow).
            nc.sync.dma_start(wk_sb[:], wk.ap())
            nc.scalar.dma_start(wv_sb[:], wv.ap())
            # xt chunks: own-row halves first; per-chunk tiles for precise
            # DMA -> PE gating; two HWDGE rings.  wq rides the scalar ring
            # behind the own-half chunks (Q runs between the KV halves).
            # chunks 0 and 1 split across BOTH rings so the first
            # projection matmuls start as early as possible
            half_k = kcols // 2
            for c in (0, 1):
                e0, e1 = (nc.sync, nc.scalar) if c == 0 else (nc.scalar,
                                                              nc.sync)
                e0.dma_start(xt_t[c][:, 0:half_k],
                             xt.ap()[c * P:(c + 1) * P, 0:half_k])
                e1.dma_start(xt_t[c][:, half_k:kcols],
                             xt.ap()[c * P:(c + 1) * P, half_k:kcols])
            for c in range(2, emb_c):
                eng = nc.sync if c % 2 == 0 else nc.scalar
                eng.dma_start(xt_t[c][:, 0:kcols],
                              xt.ap()[c * P:(c + 1) * P, 0:kcols])
            nc.scalar.dma_start(wq_sb[:], wq.ap())
            for c in range(emb_c):
                eng = nc.sync if c % 2 == 0 else nc.scalar
                eng.dma_start(xt_t[c][:, kcols:xcols],
                              xt.ap()[c * P:(c + 1) * P, kcols:xcols])
            # constants / masks (after the weight DMAs)
            nc.sync.dma_start(pad_sb[:], pad.ap())
            nc.gpsimd.memset(ones_sb[:], 1.0)
            nc.gpsimd.memset(dmask[:], 1.0)
            for j in range(sub):
                nc.gpsimd.affine_select(
                    out=dmask[:, j, :],
                    in_=dmask[:, j, :],
                    compare_op=mybir.AluOpType.is_ge,
                    fill=0.0,
                    base=-(j * P),
                    pattern=[[1, blk]],
                    channel_multiplier=-1,
                )

            # ---- K^T/V^T (both halves) and Q^T projections inside ONE
            # long-lived PSUM pool: per-bank tag reuse gives precise WAR
            # deps (next user of a bank waits only that bank's copy, not a
            # full pool-close barrier).  Chunk-outer so the PE trails the
            # xt DMA arrivals; V^T -> natural V via the DMA crossbar.
            with tc.tile_pool(name="proj_psum", bufs=1,
                              space="PSUM") as pp:
                def kv_pass(half):
                    lo = half * kcols
                    k_ps = [pp.tile([P, blk], F32, tag=f"pk{n}",
                                    name=f"kps{half}_{n}")
                            for n in range(nch)]
                    v_ps = [pp.tile([P, blk], F32, tag=f"pv{n}",
                                    name=f"vps{half}_{n}")
                            for n in range(nch)]
                    for c in range(emb_c):
                        for n in range(nch):
                            nc.tensor.matmul(
                                k_ps[n][:], wk_sb[:, c, :],
                                xt_t[c][:, lo + n * blk:lo + (n + 1) * blk],
                                start=(c == 0), stop=(c == emb_c - 1))
                        for n in range(nch):
                            nc.tensor.matmul(
                                v_ps[n][:], wv_sb[:, c, :],
                                xt_t[c][:, lo + n * blk:lo + (n + 1) * blk],
                                start=(c == 0), stop=(c == emb_c - 1))
                    # PSUM -> SBUF copies spread over two engines so the
                    # bank-release chain is not serialized on the DVE
                    for n in range(nch):
                        dst = kt_sb[:, lo + n * blk:lo + (n + 1) * blk]
                        if n % 2 == 0:
                            nc.scalar.copy(dst, k_ps[n][:])
                        else:
                            nc.vector.tensor_copy(dst, k_ps[n][:])
                    for n in range(nch):
                        t = half * nch + n
                        dst = vt_all[:, t * blk:(t + 1) * blk]
                        if n % 2 == 0:
                            nc.scalar.copy(dst, v_ps[n][:])
                        else:
                            nc.vector.tensor_copy(dst, v_ps[n][:])
                        nc.sync.dma_start_transpose(
                            v_sb[:, t * sub:(t + 1) * sub, :], dst)

                kv_pass(0)
                # Q^T reuses the K banks (precise per-bank WAR deps); it
                # needs only own-row chunks, filling the peer-chunk wait.
                q_ps = [pp.tile([P, blk], F32, tag=f"pk{n}",
                                name=f"qps_{n}")
                        for n in range(nch)]
                for n in range(nch):
                    for c in range(emb_c):
                        nc.tensor.matmul(q_ps[n][:], wq_sb[:, c, :],
                                         xt_t[c][:, n * blk:(n + 1) * blk],
                                         start=(c == 0),
                                         stop=(c == emb_c - 1))
                for n in range(nch):
                    dst = qt_sb[:, n * blk:(n + 1) * blk]
                    if n % 2 == 0:
                        nc.scalar.copy(dst, q_ps[n][:])
                    else:
                        nc.vector.tensor_copy(dst, q_ps[n][:])
                kv_pass(1)

            # ---- attention: one pass per q-tile, biggest tile first ----
            halves = 2
            hs = sub // halves
            with (
                tc.tile_pool(name="st_psum", bufs=2, space="PSUM") as stp,
                tc.tile_pool(name="ot_psum", bufs=2, space="PSUM") as otp,
                tc.tile_pool(name="sum_psum", bufs=2, space="PSUM") as smp,
                tc.tile_pool(name="pt_pool", bufs=4) as ptp,
                tc.tile_pool(name="acc_pool", bufs=4) as accp,
                tc.tile_pool(name="ot_sb_pool", bufs=2) as osp,
            ):
                for i in (3, 2, 1, 0):
                    slots = list(range(0, i + 1)) + list(range(4, 5 + i))
                    ot = otp.tile([P, blk], F32, tag="ot", name=f"ot_{i}")
                    sm = smp.tile([1, blk], F32, tag="sm", name=f"sm_{i}")
                    n_mm = 2 * (i + 1) * sub
                    mm = 0
                    qs = qt_sb[:, i * blk:(i + 1) * blk]
                    accs = []
                    sm_n = 0
                    # per-tile sum batching: fewer ones-matmuls on the PE,
                    # tree-reduced on the DVE; last tile stays unbatched so
                    # the kernel tail is short
                    plan = {3: [4, 4], 2: [4, 2], 1: [4], 0: [1, 1]}[i]
                    bi = 0
                    for si, s in enumerate(slots):
                        pts = []
                        diag = (s == i)
                        for h in range(halves):
                            st = stp.tile([P, hs * blk], F32, tag="st")
                            for j in range(hs):
                                jj = h * hs + j
                                # on the diagonal group only q >= key is
                                # live; skip the upper-triangle columns
                                off = jj * P if diag else 0
                                nc.tensor.matmul(
                                    st[:, j * blk + off:(j + 1) * blk],
                                    kt_sb[:, s * blk + jj * P:
                                          s * blk + (jj + 1) * P],
                                    qs[:, off:blk],
                                    start=True, stop=True)
                            pt = ptp.tile([P, hs * blk], BF16, tag="pt")
                            if diag:
                                for j in range(hs):
                                    off = (h * hs + j) * P
                                    nc.scalar.activation(
                                        pt[:, j * blk + off:(j + 1) * blk],
                                        st[:, j * blk + off:(j + 1) * blk],
                                        mybir.ActivationFunctionType.Exp,
                                        bias=0.0, scale=SCALE)
                                # the tri mask also zeroes the stale
                                # (skipped) upper-triangle region of pt
                                nc.vector.tensor_tensor(
                                    pt[:], pt[:],
                                    dmask[:, h * hs:(h + 1) * hs, :]
                                    .rearrange("p s b -> p (s b)"),
                                    mybir.AluOpType.mult)
                            else:
                                nc.scalar.activation(
                                    pt[:], st[:],
                                    mybir.ActivationFunctionType.Exp,
                                    bias=0.0, scale=SCALE)
                            if s == 4 + i:  # pad group (zeroed on light)
                                nc.vector.tensor_scalar_mul(
                                    pt[:], pt[:], pad_sb[:, 0:1])
                            pts.append(pt)
                        # Ot accumulation AFTER both halves' St+exp: keeps
                        # the scalar engine's exp stream gapless (exp h1
                        # only needs St h1, not AV h0)
                        for h in range(halves):
                            pt = pts[h]
                            for j in range(hs):
                                jj = h * hs + j
                                off = jj * P if diag else 0
                                nc.tensor.matmul(
                                    ot[:, off:blk],
                                    v_sb[:, s * sub + jj, :],
                                    pt[:, j * blk + off:(j + 1) * blk],
                                    start=(mm == 0),
                                    stop=(mm == n_mm - 1))
                                mm += 1
                        # row sums: DVE partial adds reduce each group to a
                        # [P, blk] tile; consecutive group PAIRS share one
                        # ones-matmul (PSUM-accumulated across the tile)
                        acc = accp.tile([P, blk], BF16, tag="acc")
                        h0, h1 = pts
                        nc.vector.tensor_tensor(
                            acc[:], h0[:, 0:blk], h0[:, blk:2 * blk],
                            mybir.AluOpType.add)
                        tmp = accp.tile([P, blk], BF16, tag="acc2")
                        nc.vector.tensor_tensor(
                            tmp[:], h1[:, 0:blk], h1[:, blk:2 * blk],
                            mybir.AluOpType.add)
                        nc.vector.tensor_tensor(
                            acc[:], acc[:], tmp[:], mybir.AluOpType.add)
                        accs.append(acc)
                        if len(accs) == plan[bi]:
                            for a in accs[1:]:
                                nc.vector.tensor_tensor(
                                    accs[0][:], accs[0][:], a[:],
                                    mybir.AluOpType.add)
                            nc.tensor.matmul(sm[:], ones_sb[:, 0:1],
                                             accs[0][:],
                                             start=(sm_n == 0),
                                             stop=(si == len(slots) - 1))
                            sm_n += 1
                            bi += 1
                            accs = []
                    nc.vector.tensor_copy(
                        sums_sb[0:1, i * blk:(i + 1) * blk], sm[:])
                    ot_out = osp.tile([P, blk], F32, tag="ot_sb")
                    if i == 0:
                        hb = blk // 2
                        nc.scalar.copy(ot_out[:, 0:hb], ot[:, 0:hb])
                        nc.sync.dma_start(
                            out_o.ap()[:, i * blk:i * blk + hb],
                            ot_out[:, 0:hb])
                        nc.vector.tensor_copy(ot_out[:, hb:blk],
                                              ot[:, hb:blk])
                        nc.sync.dma_start(
                            out_o.ap()[:, i * blk + hb:(i + 1) * blk],
                            ot_out[:, hb:blk])
                    else:
                        nc.vector.tensor_copy(ot_out[:], ot[:])
                        nc.sync.dma_start(
                            out_o.ap()[:, i * blk:(i + 1) * blk], ot_out[:])
                    nc.sync.dma_start(out_s.ap()[i:i + 1, :],
                                      sums_sb[0:1, i * blk:(i + 1) * blk])

    nc.compile()
    return nc


_NC_CACHE = {}


def _get_nc(seq: int):
    if seq not in _NC_CACHE:
        _NC_CACHE[seq] = build_nc(seq)
    return _NC_CACHE[seq]


def make_in_maps(x, Wq, Wk, Wv, seq=None):
    """Host-side sharding: build the 8 per-core input maps."""
    x = np.asarray(x, dtype=np.float32)
    Wq = np.asarray(Wq, dtype=np.float32)
    Wk = np.asarray(Wk, dtype=np.float32)
    Wv = np.asarray(Wv, dtype=np.float32)
    seq = seq or x.shape[1]
    blk = seq // NBLK
    in_maps = []

    def warr(W):
        # [1024, 128] -> [P, emb_chunks, 128] so the device DMA is contiguous
        return np.ascontiguousarray(
            W.reshape(-1, P, DK).transpose(1, 0, 2)).astype(ml_dtypes.bfloat16)

    warrs = {"wq": warr(Wq), "wk": warr(Wk), "wv": warr(Wv)}
    for core in range(NCORES):
        b, h = core // 2, core % 2
        blocks = HEAVY_BLOCKS if h == 0 else LIGHT_BLOCKS
        rows = np.concatenate(
            [np.arange(g * blk, (g + 1) * blk) for g in blocks])
        peer_blocks = LIGHT_BLOCKS if h == 0 else HEAVY_BLOCKS
        rows_peer = np.concatenate(
            [np.arange(g * blk, (g + 1) * blk) for g in peer_blocks])
        all_rows = np.concatenate([rows, rows_peer])
        xt = np.ascontiguousarray(x[b].T[:, all_rows]).astype(
            ml_dtypes.bfloat16)
        padv = np.full((P, 1), 1.0 if h == 0 else 0.0, dtype=np.float32)
        in_maps.append({
            "xt": xt,
            "pad": padv,
            **warrs,
        })
    return in_maps


def unshard(results, seq=None, batch=BATCH):
    seq = seq or SEQ
    blk = seq // NBLK
    out = np.empty((batch, seq, DK), dtype=np.float32)
    for core in range(NCORES):
        b, h = core // 2, core % 2
        blocks = HEAVY_BLOCKS if h == 0 else LIGHT_BLOCKS
        oo = np.asarray(results[core]["out_o"])  # [128, 4*blk]
        ss = np.asarray(results[core]["out_s"])  # [4, blk]
        for i, g in enumerate(blocks):
            o_cols = oo[:, i * blk:(i + 1) * blk]        # [dv, blk]
            out[b, g * blk:(g + 1) * blk, :] = (o_cols / ss[i][None, :]).T
    return out


LAST_EXEC_NS = None
LAST_RESULTS = None


def kernel(x, Wq, Wk, Wv):
    global LAST_EXEC_NS, LAST_RESULTS
    x = np.asarray(x, dtype=np.float32)
    seq = x.shape[1]
    nc = _get_nc(seq)
    in_maps = make_in_maps(x, Wq, Wk, Wv, seq)
    trace = bool(os.environ.get("BASS_KERNEL_TRACE"))
    res = run_bass_kernel_spmd(nc, in_maps, core_ids=list(range(NCORES)),
                               trace=trace)
    LAST_EXEC_NS = res.exec_time_ns
    LAST_RESULTS = res
    return unshard(res.results, seq, x.shape[0])


if __name__ == "__main__":
    rng = np.random.default_rng(0)
    x = rng.standard_normal((BATCH, SEQ, EMB), dtype=np.float32)
    Wq = rng.standard_normal((EMB, DK), dtype=np.float32) / 32
    Wk = rng.standard_normal((EMB, DK), dtype=np.float32) / 32
    Wv = rng.standard_normal((EMB, DK), dtype=np.float32) / 32
    out = kernel(x, Wq, Wk, Wv)
    print("out", out.shape, out.dtype, "exec_ns", LAST_EXEC_NS)
